# revision 3
# baseline (speedup 1.0000x reference)
"""Self-attention (CrossAttention with context=x) Bass kernel for Trainium2, 8 NeuronCores.

Problem: x:(4,2048,1024) fp32, 16 heads x 64 dim, Wq/Wk/Wv:(1024,1024), Wout:(1024,1024), bout:(1024,)
  q = x@WqT, k = x@WkT, v = x@WvT  (per head d=64, scale d**-0.25 on q and k)
  weight = softmax(q k^T), a = weight @ v, out = a@WoutT + bout

Sharding: core j handles batch j//2, head-group j%2 (8 of 16 heads).
Each core computes a partial output (its heads' contribution to out[batch]);
the host sums core pairs and adds bout.

Device-side math (per core, bf16 compute, fp32 accumulate):
  xT[c,n] (host pre-transposed), wqT/wkT/wvT[c,hd] (host pre-transposed, scale folded)
  qT[hd,n] = wqT.T @ xT      kT[hd,n] = wkT.T @ xT     v[n,hd] = xT.T @ wvT
  per head h: ST[j,i] = kT_h.T-slice matmuls (K=64); PT = exp(ST) (no max-sub:
  logits are small by construction); aT_aug[65,i] accumulates [v_h | 1].T @ PT over j
  -> rows 0..63 = unnormalized a^T, row 64 = softmax denominator Z.
  aT_scaled = aT * (1/Z) broadcast; out[i,c] = sum_h aT_scaled WoutT-slice matmuls.
"""
import os
import sys
import types
import numpy as np

import concourse.bass as bass
import concourse.bacc as bacc
import concourse.tile as tile
from concourse import mybir
from concourse import bass_utils

BF16 = mybir.dt.bfloat16
FP32 = mybir.dt.float32
NP_BF16 = mybir.dt.np(BF16)

N_CORES = 8
LAST_EXEC_TIME_NS = None


def _install_ntff_shim():
    """Shim for missing antenv.axon_hooks so trace=True can capture NTFF profiles."""
    if "antenv.axon_hooks" in sys.modules:
        return
    try:
        import antenv  # noqa: F401
        from trn_agent_boot.trn_boot import _ntff_profile_via_ctypes
    except Exception:
        return
    hook = _ntff_profile_via_ctypes("/opt/axon/libaxon_pjrt.so")
    mod = types.ModuleType("antenv.axon_hooks")
    mod._hook = hook
    mod.set_axon_ntff_profile_hook = lambda h: setattr(mod, "_hook", h)
    mod.get_axon_ntff_profile_hook = lambda: mod._hook
    sys.modules["antenv.axon_hooks"] = mod
    sys.modules["antenv"].axon_hooks = mod


def build_nc(n=2048, c=1024, hd_l=512, num_devices=N_CORES):
    """Build the per-core SPMD Bass graph.

    n: sequence length, c: model dim, hd_l: local head-dims (heads_l*64).
    """
    d = 64
    heads_l = hd_l // d
    kc_x = c // 128      # contraction chunks over c
    mt_q = hd_l // 128   # qT/kT partition tiles
    n_ich = n // 512     # i chunks of 512
    n_jt = n // 128      # j tiles of 128
    kc_w = hd_l // 128   # out-proj contraction chunks
    n_cch = c // 512     # out-proj N chunks
    n_nt = n // 128      # v n-tiles

    nc = bacc.Bacc("TRN2", target_bir_lowering=False, debug=False,
                   num_devices=num_devices)
    xT = nc.dram_tensor("xT", [c, n], BF16, kind="ExternalInput").ap()
    wqT = nc.dram_tensor("wqT", [c, hd_l], BF16, kind="ExternalInput").ap()
    wkT = nc.dram_tensor("wkT", [c, hd_l], BF16, kind="ExternalInput").ap()
    wvT = nc.dram_tensor("wvT", [c, hd_l], BF16, kind="ExternalInput").ap()
    woutT = nc.dram_tensor("woutT", [hd_l, c], BF16, kind="ExternalInput").ap()
    out = nc.dram_tensor("out", [n, c], FP32, kind="ExternalOutput").ap()

    xT_r = xT.rearrange("(kc p) n -> kc p n", p=128)
    wqT_r = wqT.rearrange("(kc p) h -> kc p h", p=128)
    wkT_r = wkT.rearrange("(kc p) h -> kc p h", p=128)
    wvT_r = wvT.rearrange("(kc p) h -> kc p h", p=128)
    woutT_r = woutT.rearrange("(kc p) c2 -> kc p c2", p=128)

    with tile.TileContext(nc) as tc:
        consts = tc.alloc_tile_pool(name="consts", bufs=1)
        # resident SBUF tensors
        xT_sb = consts.tile([128, kc_x, n], BF16, name="xT_sb")
        wqT_sb = consts.tile([128, kc_x, hd_l], BF16, name="wqT_sb")
        wkT_sb = consts.tile([128, kc_x, hd_l], BF16, name="wkT_sb")
        wvT_sb = consts.tile([128, kc_x, hd_l], BF16, name="wvT_sb")
        woutT_sb = consts.tile([128, kc_w, c], BF16, name="woutT_sb")
        qT_sb = consts.tile([128, mt_q, n], BF16, name="qT_sb")
        kT_sb = consts.tile([128, mt_q, n], BF16, name="kT_sb")
        v_aug = consts.tile([128, n_nt, heads_l, 65], BF16, name="v_aug")
        atsc = [consts.tile([128, n], BF16, name=f"atsc{i}", tag=f"atsc{i}")
                for i in range(mt_q)]

        for kc in range(kc_x):
            nc.sync.dma_start(out=xT_sb[:, kc, :], in_=xT_r[kc])
            nc.sync.dma_start(out=wqT_sb[:, kc, :], in_=wqT_r[kc])
            nc.sync.dma_start(out=wkT_sb[:, kc, :], in_=wkT_r[kc])
            nc.sync.dma_start(out=wvT_sb[:, kc, :], in_=wvT_r[kc])
        for kc in range(kc_w):
            nc.sync.dma_start(out=woutT_sb[:, kc, :], in_=woutT_r[kc])

        # ones columns of v_aug
        nc.vector.memset(v_aug[:, :, :, 64:65], 1.0)

        # ---- phase 1: q/k/v projections ----
        with tc.tile_pool(name="qkvp", bufs=4, space="PSUM") as qkvp:
            for mt in range(mt_q):
                for ich in range(n_ich):
                    q_ps = qkvp.tile([128, 512], FP32, name="q_ps", tag="qkv")
                    for kc in range(kc_x):
                        nc.tensor.matmul(
                            q_ps[:, :],
                            lhsT=wqT_sb[:, kc, mt * 128:(mt + 1) * 128],
                            rhs=xT_sb[:, kc, ich * 512:(ich + 1) * 512],
                            start=(kc == 0), stop=(kc == kc_x - 1))
                    nc.vector.tensor_copy(
                        qT_sb[:, mt, ich * 512:(ich + 1) * 512], q_ps[:, :])
                    k_ps = qkvp.tile([128, 512], FP32, name="k_ps", tag="qkv")
                    for kc in range(kc_x):
                        nc.tensor.matmul(
                            k_ps[:, :],
                            lhsT=wkT_sb[:, kc, mt * 128:(mt + 1) * 128],
                            rhs=xT_sb[:, kc, ich * 512:(ich + 1) * 512],
                            start=(kc == 0), stop=(kc == kc_x - 1))
                    nc.vector.tensor_copy(
                        kT_sb[:, mt, ich * 512:(ich + 1) * 512], k_ps[:, :])
            for nt in range(n_nt):
                v_ps = qkvp.tile([128, hd_l], FP32, name="v_ps", tag="qkv")
                for kc in range(kc_x):
                    nc.tensor.matmul(
                        v_ps[:, :],
                        lhsT=xT_sb[:, kc, nt * 128:(nt + 1) * 128],
                        rhs=wvT_sb[:, kc, :],
                        start=(kc == 0), stop=(kc == kc_x - 1))
                nc.vector.tensor_copy(
                    v_aug[:, nt, :, 0:64],
                    v_ps.rearrange("p (h e) -> p h e", e=64))

        # ---- phase 2: attention per local head ----
        with tc.tile_pool(name="stp", bufs=1, space="PSUM") as stp, \
             tc.tile_pool(name="atp", bufs=1, space="PSUM") as atp, \
             tc.tile_pool(name="ptp", bufs=3) as ptp, \
             tc.tile_pool(name="zp", bufs=2) as zp:
            for h in range(heads_l):
                mt = h // 2
                ro = (h % 2) * 64  # row offset within the mt tile
                at_ps = atp.tile([65, n], FP32, name="at_ps", tag="at")
                for jt in range(n_jt):
                    st = stp.tile([128, n], FP32, name="st", tag="st")
                    for ich in range(n_ich):
                        nc.tensor.matmul(
                            st[:, ich * 512:(ich + 1) * 512],
                            lhsT=kT_sb[ro:ro + 64, mt, jt * 128:(jt + 1) * 128],
                            rhs=qT_sb[ro:ro + 64, mt, ich * 512:(ich + 1) * 512],
                            start=True, stop=True)
                    pt = ptp.tile([128, n], BF16, name="pt", tag="pt")
                    nc.scalar.activation(pt[:, :], st[:, :],
                                         mybir.ActivationFunctionType.Exp)
                    for ich in range(n_ich):
                        nc.tensor.matmul(
                            at_ps[:, ich * 512:(ich + 1) * 512],
                            lhsT=v_aug[:, jt, h, :],
                            rhs=pt[:, ich * 512:(ich + 1) * 512],
                            start=(jt == 0), stop=(jt == n_jt - 1))
                # normalize: rows 0..63 scaled by 1/Z (row 64)
                rz = zp.tile([1, n], FP32, name="rz", tag="rz")
                nc.vector.reciprocal(rz[:, :], at_ps[64:65, :])
                zb = zp.tile([64, n], FP32, name="zb", tag="zb")
                nc.gpsimd.partition_broadcast(zb[:, :], rz[:, :])
                nc.vector.tensor_mul(atsc[mt][ro:ro + 64, :], at_ps[0:64, :],
                                     zb[:, :])

        # ---- phase 3: output projection ----
        with tc.tile_pool(name="opp", bufs=2, space="PSUM") as opp, \
             tc.tile_pool(name="osb", bufs=3) as osb:
            for it in range(n // 128):
                o_ps = opp.tile([128, c], FP32, name="o_ps", tag="o")
                for cch in range(n_cch):
                    for kc in range(kc_w):
                        nc.tensor.matmul(
                            o_ps[:, cch * 512:(cch + 1) * 512],
                            lhsT=atsc[kc][:, it * 128:(it + 1) * 128],
                            rhs=woutT_sb[:, kc, cch * 512:(cch + 1) * 512],
                            start=(kc == 0), stop=(kc == kc_w - 1))
                o_sb = osb.tile([128, c], FP32, name="o_sb", tag="osb")
                nc.vector.tensor_copy(o_sb[:, :], o_ps[:, :])
                nc.sync.dma_start(out=out[it * 128:(it + 1) * 128, :],
                                  in_=o_sb[:, :])
        consts.release()

    nc.compile()
    return nc


def make_in_maps(x, Wq, Wk, Wv, Wout, n=2048, c=1024, heads=16, d=64):
    """Shard + pre-transpose + cast inputs for the 8 cores."""
    s = float(d) ** -0.25
    hd_l = (heads // 2) * d
    wT = {}
    for g in range(2):
        sl = slice(g * hd_l, (g + 1) * hd_l)
        wT[g] = (
            np.ascontiguousarray((Wq[sl] * s).T).astype(NP_BF16),
            np.ascontiguousarray((Wk[sl] * s).T).astype(NP_BF16),
            np.ascontiguousarray(Wv[sl].T).astype(NP_BF16),
            np.ascontiguousarray(Wout.T[sl]).astype(NP_BF16),
        )
    in_maps = []
    for core in range(N_CORES):
        b = core // 2
        g = core % 2
        wq, wk, wv, wo = wT[g]
        in_maps.append({
            "xT": np.ascontiguousarray(x[b].T).astype(NP_BF16),
            "wqT": wq, "wkT": wk, "wvT": wv, "woutT": wo,
        })
    return in_maps


_NC_CACHE = {}


def kernel(x, Wq, Wk, Wv, Wout, bout):
    global LAST_EXEC_TIME_NS
    b, n, c = x.shape
    heads = 16
    d = 64
    hd_l = (heads // 2) * d

    if "nc" not in _NC_CACHE:
        _NC_CACHE["nc"] = build_nc(n=n, c=c, hd_l=hd_l)
    nc = _NC_CACHE["nc"]

    in_maps = make_in_maps(np.asarray(x, np.float32), np.asarray(Wq, np.float32),
                           np.asarray(Wk, np.float32), np.asarray(Wv, np.float32),
                           np.asarray(Wout, np.float32), n=n, c=c, heads=heads, d=d)

    profile = os.environ.get("BASS_KERNEL_PROFILE", "0") == "1"
    if profile:
        _install_ntff_shim()
    res = bass_utils.run_bass_kernel_spmd(
        nc, in_maps, core_ids=list(range(N_CORES)), trace=profile)
    LAST_EXEC_TIME_NS = res.exec_time_ns

    bout = np.asarray(bout, np.float32)
    out = np.empty((b, n, c), np.float32)
    for bb in range(b):
        out[bb] = res.results[2 * bb]["out"] + res.results[2 * bb + 1]["out"] + bout
    return out


# revision 10
# speedup vs baseline: 1.5605x; 1.5605x over previous
"""Self-attention (CrossAttention with context=x) Bass kernel for Trainium2, 8 NeuronCores.

Problem: x:(4,2048,1024) fp32, 16 heads x 64 dim, Wq/Wk/Wv:(1024,1024), Wout:(1024,1024), bout:(1024,)
  q = x@WqT, k = x@WkT, v = x@WvT  (per head d=64, scale d**-0.25 on q and k)
  weight = softmax(q k^T), a = weight @ v, out = a@WoutT + bout

Sharding: core j handles batch j//2, head-group j%2 (8 of 16 heads).
Each core computes a partial output (its heads' contribution to out[batch]);
the host sums core pairs and adds bout.

V3 design (per core, bf16 compute, fp32 accumulate):
- Head-PAIR processing: scores row-tiled on the PE array (head A rows 0-63,
  head B rows 64-127, concurrent K=64 matmuls); values col-tiled (A cols 0-63,
  B cols 64-127, concurrent M=64 matmuls).
- exp on ScalarE from PSUM in [128,1024] tiles, double-buffered per head so
  ScalarE stays saturated. No max-subtraction (logits are small by construction).
- Softmax denominators: Zpart[p,i] += PT chunks on VectorE (bf16), column-summed
  by a ones-vector matmul; reciprocal computed in a 32x32-transposed layout so it
  runs wide across partitions; broadcast to rows via GpSimd partition_broadcast;
  normalization folded into a single PSUM->SBUF multiply per pair.
- Output projection from normalized a^T tiles; host adds pair partials + bout.
"""
import os
import sys
import types
import numpy as np

import concourse.bass as bass
import concourse.bacc as bacc
import concourse.tile as tile
from concourse import mybir
from concourse import bass_utils

BF16 = mybir.dt.bfloat16
FP32 = mybir.dt.float32
NP_BF16 = mybir.dt.np(BF16)

N_CORES = 8
LAST_EXEC_TIME_NS = None


def _install_ntff_shim():
    """Shim for missing antenv.axon_hooks so trace=True can capture NTFF profiles."""
    if "antenv.axon_hooks" in sys.modules:
        return
    try:
        import antenv  # noqa: F401
        from trn_agent_boot.trn_boot import _ntff_profile_via_ctypes
    except Exception:
        return
    hook = _ntff_profile_via_ctypes("/opt/axon/libaxon_pjrt.so")
    mod = types.ModuleType("antenv.axon_hooks")
    mod._hook = hook
    mod.set_axon_ntff_profile_hook = lambda h: setattr(mod, "_hook", h)
    mod.get_axon_ntff_profile_hook = lambda: mod._hook
    sys.modules["antenv.axon_hooks"] = mod
    sys.modules["antenv"].axon_hooks = mod


def build_nc(n=2048, c=1024, hd_l=512, num_devices=N_CORES, debug=False):
    """Build the per-core SPMD Bass graph.

    n: sequence length, c: model dim, hd_l: local head-dims (heads_l*64).
    """
    d = 64
    heads_l = hd_l // d
    pairs = heads_l // 2
    kc_x = c // 128      # contraction chunks over c
    mt_q = hd_l // 128   # qT/kT partition tiles (one per head pair)
    n_ich = n // 512     # i chunks of 512
    n_jt = n // 128      # j tiles of 128
    kc_w = hd_l // 128   # out-proj contraction chunks
    n_cch = c // 512     # out-proj N chunks
    n_ih = n // 1024     # i halves

    nc = bacc.Bacc("TRN2", target_bir_lowering=False, debug=False,
                   num_devices=num_devices)
    xT = nc.dram_tensor("xT", [c, n], BF16, kind="ExternalInput").ap()
    wqT = nc.dram_tensor("wqT", [c, hd_l], BF16, kind="ExternalInput").ap()
    wkT = nc.dram_tensor("wkT", [c, hd_l], BF16, kind="ExternalInput").ap()
    wvT = nc.dram_tensor("wvT", [c, hd_l], BF16, kind="ExternalInput").ap()
    woutT = nc.dram_tensor("woutT", [hd_l, c], BF16, kind="ExternalInput").ap()
    out = nc.dram_tensor("out", [n, c], FP32, kind="ExternalOutput").ap()
    if debug:
        dbg_at = nc.dram_tensor("dbg_at", [128, n], FP32, kind="ExternalOutput").ap()
        dbg_zp = nc.dram_tensor("dbg_zp", [128, n], FP32, kind="ExternalOutput").ap()
        dbg_zd = nc.dram_tensor("dbg_zd", [64, n], FP32, kind="ExternalOutput").ap()
        dbg_zback = nc.dram_tensor("dbg_zback", [64, n], FP32, kind="ExternalOutput").ap()
        dbg_zb = nc.dram_tensor("dbg_zb", [128, n], FP32, kind="ExternalOutput").ap()

    xT_r = xT.rearrange("(kc p) n -> kc p n", p=128)
    wqT_r = wqT.rearrange("(kc p) h -> kc p h", p=128)
    wkT_r = wkT.rearrange("(kc p) h -> kc p h", p=128)
    wvT_r = wvT.rearrange("(kc p) h -> kc p h", p=128)
    woutT_r = woutT.rearrange("(kc p) c2 -> kc p c2", p=128)

    with tile.TileContext(nc) as tc:
        persist = tc.alloc_tile_pool(name="persist", bufs=1)
        qT_sb = persist.tile([128, mt_q, n], BF16, name="qT_sb")
        kT_sb = persist.tile([128, mt_q, n], BF16, name="kT_sb")
        v_sb = persist.tile([128, n_jt, hd_l], BF16, name="v_sb")
        woutT_sb = persist.tile([128, kc_w, c], BF16, name="woutT_sb")
        atsc = [persist.tile([128, n], BF16, name=f"atsc{i}", tag=f"atsc{i}")
                for i in range(mt_q)]
        ones_sb = persist.tile([128, 1], BF16, name="ones_sb")
        nc.vector.memset(ones_sb[:, :], 1.0)

        ph1 = tc.alloc_tile_pool(name="ph1", bufs=1)
        xT_sb = ph1.tile([128, kc_x, n], BF16, name="xT_sb")
        wqT_sb = ph1.tile([128, kc_x, hd_l], BF16, name="wqT_sb")
        wkT_sb = ph1.tile([128, kc_x, hd_l], BF16, name="wkT_sb")
        wvT_sb = ph1.tile([128, kc_x, hd_l], BF16, name="wvT_sb")

        for kc in range(kc_x):
            nc.sync.dma_start(out=xT_sb[:, kc, :], in_=xT_r[kc])
            nc.sync.dma_start(out=wqT_sb[:, kc, :], in_=wqT_r[kc])
            nc.sync.dma_start(out=wkT_sb[:, kc, :], in_=wkT_r[kc])
            nc.sync.dma_start(out=wvT_sb[:, kc, :], in_=wvT_r[kc])
        for kc in range(kc_w):
            nc.sync.dma_start(out=woutT_sb[:, kc, :], in_=woutT_r[kc])

        # ---- phase 1: q/k/v projections ----
        # kc outer / ich inner: one weight load per 4 streamed chunks.
        with tc.tile_pool(name="qkvp", bufs=8, space="PSUM") as qkvp:
            for mt in range(mt_q):
                for wsb, dst in ((wqT_sb, qT_sb), (wkT_sb, kT_sb)):
                    ps = [qkvp.tile([128, 512], FP32, name=f"ps{i}", tag="qkv")
                          for i in range(n_ich)]
                    for kc in range(kc_x):
                        for ich in range(n_ich):
                            nc.tensor.matmul(
                                ps[ich][:, :],
                                lhsT=wsb[:, kc, mt * 128:(mt + 1) * 128],
                                rhs=xT_sb[:, kc, ich * 512:(ich + 1) * 512],
                                start=(kc == 0), stop=(kc == kc_x - 1))
                    for ich in range(n_ich):
                        nc.vector.tensor_copy(
                            dst[:, mt, ich * 512:(ich + 1) * 512], ps[ich][:, :])
            for nt in range(n_jt):
                v_ps = qkvp.tile([128, hd_l], FP32, name="v_ps", tag="qkv")
                for kc in range(kc_x):
                    nc.tensor.matmul(
                        v_ps[:, :],
                        lhsT=xT_sb[:, kc, nt * 128:(nt + 1) * 128],
                        rhs=wvT_sb[:, kc, :],
                        start=(kc == 0), stop=(kc == kc_x - 1))
                nc.vector.tensor_copy(v_sb[:, nt, :], v_ps[:, :])
        ph1.release()

        # ---- phase 2: attention, one head pair at a time ----
        stp = tc.alloc_tile_pool(name="stp", bufs=2, space="PSUM")
        atp = tc.alloc_tile_pool(name="atp", bufs=1, space="PSUM")
        ptp = tc.alloc_tile_pool(name="ptp", bufs=14)
        zpp = tc.alloc_tile_pool(name="zpp", bufs=2)
        zdp = tc.alloc_tile_pool(name="zdp", bufs=1)
        zbp = tc.alloc_tile_pool(name="zbp", bufs=2)
        zdramp = tc.alloc_tile_pool(name="zdramp", bufs=2, space="DRAM")

        zdance = zdp.tile([64, n], FP32, name="zdance")
        zdance_r = zdp.tile([64, n], FP32, name="zdance_r")
        zback = zdp.tile([64, n], FP32, name="zback")
        nc.vector.memset(zdance[:, :], 1.0)
        nc.vector.memset(zdance_r[:, :], 1.0)

        for pt in range(pairs):
            roA, roB = 0, 64
            at = atp.tile([128, n], FP32, name="at", tag="at")
            zpA = zpp.tile([128, n], BF16, name="zpA", tag="zpA")
            zpB = zpp.tile([128, n], BF16, name="zpB", tag="zpB")
            for jt in range(n_jt):
                pts = []
                for ih in range(n_ih):
                    stA = stp.tile([128, 1024], FP32, name="stA", tag="st")
                    stB = stp.tile([128, 1024], FP32, name="stB", tag="st")
                    for s2 in range(2):  # two 512-chunks per half
                        ich = ih * 2 + s2
                        nc.tensor.matmul(
                            stA[:, s2 * 512:(s2 + 1) * 512],
                            lhsT=kT_sb[roA:roA + 64, pt, jt * 128:(jt + 1) * 128],
                            rhs=qT_sb[roA:roA + 64, pt, ich * 512:(ich + 1) * 512],
                            start=True, stop=True)
                        nc.tensor.matmul(
                            stB[:, s2 * 512:(s2 + 1) * 512],
                            lhsT=kT_sb[roB:roB + 64, pt, jt * 128:(jt + 1) * 128],
                            rhs=qT_sb[roB:roB + 64, pt, ich * 512:(ich + 1) * 512],
                            start=True, stop=True)
                    ptA = ptp.tile([128, 1024], BF16, name="ptA", tag="pt")
                    ptB = ptp.tile([128, 1024], BF16, name="ptB", tag="pt")
                    nc.scalar.activation(ptA[:, :], stA[:, :],
                                         mybir.ActivationFunctionType.Exp)
                    nc.scalar.activation(ptB[:, :], stB[:, :],
                                         mybir.ActivationFunctionType.Exp)
                    sl = slice(ih * 1024, (ih + 1) * 1024)
                    if jt == 0:
                        nc.vector.tensor_copy(zpA[:, sl], ptA[:, :])
                        nc.vector.tensor_copy(zpB[:, sl], ptB[:, :])
                    else:
                        nc.vector.tensor_add(zpA[:, sl], zpA[:, sl], ptA[:, :])
                        nc.vector.tensor_add(zpB[:, sl], zpB[:, sl], ptB[:, :])
                    pts.append((ptA, ptB))
                # values for this j chunk (col-tiled A/B, accumulate over jt)
                vA = v_sb[:, jt, (2 * pt) * 64:(2 * pt) * 64 + 64]
                vB = v_sb[:, jt, (2 * pt + 1) * 64:(2 * pt + 1) * 64 + 64]
                for ih in range(n_ih):
                    ptA, ptB = pts[ih]
                    for s2 in range(2):
                        ich = ih * 2 + s2
                        csl = slice(ich * 512, (ich + 1) * 512)
                        psl = slice(s2 * 512, (s2 + 1) * 512)
                        nc.tensor.matmul(
                            at[roA:roA + 64, csl], lhsT=vA, rhs=ptA[:, psl],
                            start=(jt == 0), stop=(jt == n_jt - 1),
                            skip_group_check=True)
                        nc.tensor.matmul(
                            at[roB:roB + 64, csl], lhsT=vB, rhs=ptB[:, psl],
                            start=(jt == 0), stop=(jt == n_jt - 1),
                            skip_group_check=True)
            # Z = column sums of Zpart via ones-matmul, then wide reciprocal
            for hh, zp in ((0, zpA), (1, zpB)):
                for ih in range(n_ih):
                    cs = stp.tile([1, 1024], FP32, name="cs", tag="st")
                    for s2 in range(2):
                        nc.tensor.matmul(
                            cs[:, s2 * 512:(s2 + 1) * 512],
                            lhsT=ones_sb[:, :],
                            rhs=zp[:, (ih * 2 + s2) * 512:(ih * 2 + s2 + 1) * 512],
                            start=True, stop=True)
                    nc.vector.tensor_copy(
                        zdance[32 * hh:32 * hh + 1, ih * 1024:(ih + 1) * 1024],
                        cs[:, :])
            nc.vector.transpose(zdance_r[:, :], zdance[:, :])
            rview = zdance_r.rearrange("p (b r) -> p b r", r=32)
            nc.vector.reciprocal(rview[:, :, 0:1], rview[:, :, 0:1])
            nc.vector.transpose(zback[:, :], zdance_r[:, :])
            zb = zbp.tile([128, n], FP32, name="zb", tag="zb")
            zdA = zdramp.tile([1, n], FP32, name="zdA", tag="zdA")
            zdB = zdramp.tile([1, n], FP32, name="zdB", tag="zdB")
            nc.sync.dma_start(out=zdA[:, :], in_=zback[0:1, :])
            nc.sync.dma_start(out=zdB[:, :], in_=zback[32:33, :])
            nc.sync.dma_start(out=zb[0:64, :],
                              in_=zdA[:, :].to_broadcast((64, n)))
            nc.sync.dma_start(out=zb[64:128, :],
                              in_=zdB[:, :].to_broadcast((64, n)))
            if debug and pt == 0:
                dbg_sb = zbp.tile([128, n], FP32, name="dbg_sb", tag="dbgsb")
                nc.vector.tensor_copy(dbg_sb[:, :], at[:, :])
                nc.sync.dma_start(out=dbg_at, in_=dbg_sb[:, :])
                dbg_sb2 = zbp.tile([128, n], FP32, name="dbg_sb2", tag="dbgsb2")
                nc.vector.tensor_copy(dbg_sb2[:, :], zpA[:, :])
                nc.sync.dma_start(out=dbg_zp, in_=dbg_sb2[:, :])
                nc.sync.dma_start(out=dbg_zd, in_=zdance[:, :])
                nc.sync.dma_start(out=dbg_zback, in_=zback[:, :])
                nc.sync.dma_start(out=dbg_zb, in_=zb[:, :])
            nc.vector.tensor_mul(atsc[pt][:, :], at[:, :], zb[:, :])
        zdramp.release()
        zbp.release()
        zdp.release()
        zpp.release()
        ptp.release()
        atp.release()
        stp.release()

        # ---- phase 3: output projection ----
        with tc.tile_pool(name="opp", bufs=2, space="PSUM") as opp, \
             tc.tile_pool(name="osb", bufs=3) as osb:
            for it in range(n // 128):
                o_ps = opp.tile([128, c], FP32, name="o_ps", tag="o")
                for kc in range(kc_w):
                    for cch in range(n_cch):
                        nc.tensor.matmul(
                            o_ps[:, cch * 512:(cch + 1) * 512],
                            lhsT=atsc[kc][:, it * 128:(it + 1) * 128],
                            rhs=woutT_sb[:, kc, cch * 512:(cch + 1) * 512],
                            start=(kc == 0), stop=(kc == kc_w - 1))
                o_sb = osb.tile([128, c], FP32, name="o_sb", tag="osb")
                nc.vector.tensor_copy(o_sb[:, :], o_ps[:, :])
                nc.sync.dma_start(out=out[it * 128:(it + 1) * 128, :],
                                  in_=o_sb[:, :])
        persist.release()

    nc.compile()
    return nc


def make_in_maps(x, Wq, Wk, Wv, Wout, n=2048, c=1024, heads=16, d=64):
    """Shard + pre-transpose + cast inputs for the 8 cores."""
    s = float(d) ** -0.25
    hd_l = (heads // 2) * d
    wT = {}
    for g in range(2):
        sl = slice(g * hd_l, (g + 1) * hd_l)
        wT[g] = (
            np.ascontiguousarray((Wq[sl] * s).T).astype(NP_BF16),
            np.ascontiguousarray((Wk[sl] * s).T).astype(NP_BF16),
            np.ascontiguousarray(Wv[sl].T).astype(NP_BF16),
            np.ascontiguousarray(Wout.T[sl]).astype(NP_BF16),
        )
    in_maps = []
    for core in range(N_CORES):
        b = core // 2
        g = core % 2
        wq, wk, wv, wo = wT[g]
        in_maps.append({
            "xT": np.ascontiguousarray(x[b].T).astype(NP_BF16),
            "wqT": wq, "wkT": wk, "wvT": wv, "woutT": wo,
        })
    return in_maps


_NC_CACHE = {}


def kernel(x, Wq, Wk, Wv, Wout, bout):
    global LAST_EXEC_TIME_NS
    b, n, c = x.shape
    heads = 16
    d = 64
    hd_l = (heads // 2) * d

    if "nc" not in _NC_CACHE:
        _NC_CACHE["nc"] = build_nc(n=n, c=c, hd_l=hd_l)
    nc = _NC_CACHE["nc"]

    in_maps = make_in_maps(np.asarray(x, np.float32), np.asarray(Wq, np.float32),
                           np.asarray(Wk, np.float32), np.asarray(Wv, np.float32),
                           np.asarray(Wout, np.float32), n=n, c=c, heads=heads, d=d)

    profile = os.environ.get("BASS_KERNEL_PROFILE", "0") == "1"
    if profile:
        _install_ntff_shim()
    res = bass_utils.run_bass_kernel_spmd(
        nc, in_maps, core_ids=list(range(N_CORES)), trace=profile)
    LAST_EXEC_TIME_NS = res.exec_time_ns

    bout = np.asarray(bout, np.float32)
    out = np.empty((b, n, c), np.float32)
    for bb in range(b):
        out[bb] = res.results[2 * bb]["out"] + res.results[2 * bb + 1]["out"] + bout
    return out


# revision 12
# speedup vs baseline: 2.0218x; 1.2956x over previous
"""Self-attention (CrossAttention with context=x) Bass kernel for Trainium2, 8 NeuronCores.

Problem: x:(4,2048,1024) fp32, 16 heads x 64 dim, Wq/Wk/Wv:(1024,1024), Wout:(1024,1024), bout:(1024,)
  q = x@WqT, k = x@WkT, v = x@WvT  (per head d=64, scale d**-0.25 on q and k)
  weight = softmax(q k^T), a = weight @ v, out = a@WoutT + bout

Sharding: core j handles batch j//2, head-group j%2 (8 of 16 heads).
Each core computes a partial output (its heads' contribution to out[batch]);
the host sums core pairs and adds bout.

V3 design (per core, bf16 compute, fp32 accumulate):
- Head-PAIR processing: scores row-tiled on the PE array (head A rows 0-63,
  head B rows 64-127, concurrent K=64 matmuls); values col-tiled (A cols 0-63,
  B cols 64-127, concurrent M=64 matmuls).
- exp on ScalarE from PSUM in [128,1024] tiles, double-buffered per head so
  ScalarE stays saturated. No max-subtraction (logits are small by construction).
- Softmax denominators: Zpart[p,i] += PT chunks on VectorE (bf16), column-summed
  by a ones-vector matmul; reciprocal computed in a 32x32-transposed layout so it
  runs wide across partitions; broadcast to rows via GpSimd partition_broadcast;
  normalization folded into a single PSUM->SBUF multiply per pair.
- Output projection from normalized a^T tiles; host adds pair partials + bout.
"""
import os
import sys
import types
import numpy as np

import concourse.bass as bass
import concourse.bacc as bacc
import concourse.tile as tile
from concourse import mybir
from concourse import bass_utils

BF16 = mybir.dt.bfloat16
FP32 = mybir.dt.float32
NP_BF16 = mybir.dt.np(BF16)

N_CORES = 8
LAST_EXEC_TIME_NS = None


def _install_ntff_shim():
    """Shim for missing antenv.axon_hooks so trace=True can capture NTFF profiles."""
    if "antenv.axon_hooks" in sys.modules:
        return
    try:
        import antenv  # noqa: F401
        from trn_agent_boot.trn_boot import _ntff_profile_via_ctypes
    except Exception:
        return
    hook = _ntff_profile_via_ctypes("/opt/axon/libaxon_pjrt.so")
    mod = types.ModuleType("antenv.axon_hooks")
    mod._hook = hook
    mod.set_axon_ntff_profile_hook = lambda h: setattr(mod, "_hook", h)
    mod.get_axon_ntff_profile_hook = lambda: mod._hook
    sys.modules["antenv.axon_hooks"] = mod
    sys.modules["antenv"].axon_hooks = mod


def build_nc(n=2048, c=1024, hd_l=512, num_devices=N_CORES, debug=False):
    """Build the per-core SPMD Bass graph.

    n: sequence length, c: model dim, hd_l: local head-dims (heads_l*64).
    """
    d = 64
    heads_l = hd_l // d
    pairs = heads_l // 2
    kc_x = c // 128      # contraction chunks over c
    mt_q = hd_l // 128   # qT/kT partition tiles (one per head pair)
    n_ich = n // 512     # i chunks of 512
    n_jt = n // 128      # j tiles of 128
    kc_w = hd_l // 128   # out-proj contraction chunks
    n_cch = c // 512     # out-proj N chunks
    n_ih = n // 1024     # i halves

    nc = bacc.Bacc("TRN2", target_bir_lowering=False, debug=False,
                   num_devices=num_devices)
    xT = nc.dram_tensor("xT", [c, n], BF16, kind="ExternalInput").ap()
    wqT = nc.dram_tensor("wqT", [c, hd_l], BF16, kind="ExternalInput").ap()
    wkT = nc.dram_tensor("wkT", [c, hd_l], BF16, kind="ExternalInput").ap()
    wvT = nc.dram_tensor("wvT", [c, hd_l], BF16, kind="ExternalInput").ap()
    woutT = nc.dram_tensor("woutT", [hd_l, c], BF16, kind="ExternalInput").ap()
    out = nc.dram_tensor("out", [n, c], FP32, kind="ExternalOutput").ap()
    if debug:
        dbg_at = nc.dram_tensor("dbg_at", [128, n], FP32, kind="ExternalOutput").ap()
        dbg_zp = nc.dram_tensor("dbg_zp", [128, n], FP32, kind="ExternalOutput").ap()
        dbg_zd = nc.dram_tensor("dbg_zd", [64, n], FP32, kind="ExternalOutput").ap()
        dbg_zback = nc.dram_tensor("dbg_zback", [64, n], FP32, kind="ExternalOutput").ap()
        dbg_zb = nc.dram_tensor("dbg_zb", [128, n], FP32, kind="ExternalOutput").ap()

    xT_r = xT.rearrange("(kc p) n -> kc p n", p=128)
    wqT_r = wqT.rearrange("(kc p) h -> kc p h", p=128)
    wkT_r = wkT.rearrange("(kc p) h -> kc p h", p=128)
    wvT_r = wvT.rearrange("(kc p) h -> kc p h", p=128)
    woutT_r = woutT.rearrange("(kc p) c2 -> kc p c2", p=128)

    with tile.TileContext(nc) as tc:
        persist = tc.alloc_tile_pool(name="persist", bufs=1)
        qT_sb = persist.tile([128, mt_q, n], BF16, name="qT_sb")
        kT_sb = persist.tile([128, mt_q, n], BF16, name="kT_sb")
        v_sb = persist.tile([128, n_jt, hd_l], BF16, name="v_sb")
        woutT_sb = persist.tile([128, kc_w, c], BF16, name="woutT_sb")
        atsc = [persist.tile([128, n], BF16, name=f"atsc{i}", tag=f"atsc{i}")
                for i in range(mt_q)]
        ones_sb = persist.tile([128, 1], BF16, name="ones_sb")
        nc.vector.memset(ones_sb[:, :], 1.0)

        ph1 = tc.alloc_tile_pool(name="ph1", bufs=1)
        xT_sb = ph1.tile([128, kc_x, n], BF16, name="xT_sb")
        wqT_sb = ph1.tile([128, kc_x, hd_l], BF16, name="wqT_sb")
        wkT_sb = ph1.tile([128, kc_x, hd_l], BF16, name="wkT_sb")
        wvT_sb = ph1.tile([128, kc_x, hd_l], BF16, name="wvT_sb")

        for kc in range(kc_x):
            nc.sync.dma_start(out=xT_sb[:, kc, :], in_=xT_r[kc])
            nc.sync.dma_start(out=wqT_sb[:, kc, :], in_=wqT_r[kc])
            nc.sync.dma_start(out=wkT_sb[:, kc, :], in_=wkT_r[kc])
            nc.sync.dma_start(out=wvT_sb[:, kc, :], in_=wvT_r[kc])
        for kc in range(kc_w):
            nc.sync.dma_start(out=woutT_sb[:, kc, :], in_=woutT_r[kc])

        # ---- phases 1+2 interleaved: qkv chains feed the attention pipeline ----
        # Attention is ScalarE-bound (exp). qkv matmul chains are emitted inside
        # the attention loop so the PE computes projections while ScalarE exps.
        stp = tc.alloc_tile_pool(name="stp", bufs=2, space="PSUM")
        atp = tc.alloc_tile_pool(name="atp", bufs=1, space="PSUM")
        qkvp = tc.alloc_tile_pool(name="qkvp", bufs=2, space="PSUM")
        ptp = tc.alloc_tile_pool(name="ptp", bufs=14)
        zpp = tc.alloc_tile_pool(name="zpp", bufs=2)
        zdp = tc.alloc_tile_pool(name="zdp", bufs=2)
        zbp = tc.alloc_tile_pool(name="zbp", bufs=2)
        zdramp = tc.alloc_tile_pool(name="zdramp", bufs=2, space="DRAM")

        def qk_chain(wsb, dst, mt, ip):
            """One q/k projection chain: 2 psum tiles, LDW amortized x2."""
            ps = [qkvp.tile([128, 512], FP32, name="ps", tag="qkv")
                  for _ in range(2)]
            for kc in range(kc_x):
                for i2 in range(2):
                    ich = ip * 2 + i2
                    nc.tensor.matmul(
                        ps[i2][:, :],
                        lhsT=wsb[:, kc, mt * 128:(mt + 1) * 128],
                        rhs=xT_sb[:, kc, ich * 512:(ich + 1) * 512],
                        start=(kc == 0), stop=(kc == kc_x - 1))
            for i2 in range(2):
                ich = ip * 2 + i2
                nc.vector.tensor_copy(
                    dst[:, mt, ich * 512:(ich + 1) * 512], ps[i2][:, :])

        def v_chain(nt):
            v_ps = qkvp.tile([128, hd_l], FP32, name="v_ps", tag="qkv")
            for kc in range(kc_x):
                nc.tensor.matmul(
                    v_ps[:, :],
                    lhsT=xT_sb[:, kc, nt * 128:(nt + 1) * 128],
                    rhs=wvT_sb[:, kc, :],
                    start=(kc == 0), stop=(kc == kc_x - 1))
            nc.vector.tensor_copy(v_sb[:, nt, :], v_ps[:, :])

        # work list of deferred qkv chains, emitted inside the attention loop
        work = []
        for nt in range(2, n_jt):
            work.append(("v", nt))
        for mt in range(1, mt_q):
            for ip in range(n_ich // 2):
                work.append(("q", mt, ip))
                work.append(("k", mt, ip))
        wi = 0

        def emit_work(k_items):
            nonlocal wi
            for _ in range(k_items):
                if wi >= len(work):
                    return
                item = work[wi]
                wi += 1
                if item[0] == "v":
                    v_chain(item[1])
                else:
                    wsb, dst = ((wqT_sb, qT_sb) if item[0] == "q"
                                else (wkT_sb, kT_sb))
                    qk_chain(wsb, dst, item[1], item[2])

        # prime: q/k for pair 0 and the first two v tiles
        for ip in range(n_ich // 2):
            qk_chain(wqT_sb, qT_sb, 0, ip)
            qk_chain(wkT_sb, kT_sb, 0, ip)
        v_chain(0)
        v_chain(1)

        zdance = zdp.tile([64, 1024], FP32, name="zdance", tag="zd")
        zdance_r = zdp.tile([64, 1024], FP32, name="zdance_r", tag="zdr")
        zback = zdp.tile([64, 1024], FP32, name="zback", tag="zbk")
        nc.vector.memset(zdance[:, :], 1.0)
        nc.vector.memset(zdance_r[:, :], 1.0)

        for pt in range(pairs):
            roA, roB = 0, 64
            hA, hB = 2 * pt, 2 * pt + 1
            zpA = zpp.tile([128, n], BF16, name="zpA", tag="zpA")
            zpB = zpp.tile([128, n], BF16, name="zpB", tag="zpB")
            for ih in range(n_ih):
                at = atp.tile([128, 1024], FP32, name="at", tag="at")
                for jt in range(n_jt):
                    stA = stp.tile([128, 1024], FP32, name="stA", tag="st")
                    stB = stp.tile([128, 1024], FP32, name="stB", tag="st")
                    for s2 in range(2):
                        ich = ih * 2 + s2
                        nc.tensor.matmul(
                            stA[:, s2 * 512:(s2 + 1) * 512],
                            lhsT=kT_sb[roA:roA + 64, pt, jt * 128:(jt + 1) * 128],
                            rhs=qT_sb[roA:roA + 64, pt, ich * 512:(ich + 1) * 512],
                            start=True, stop=True)
                        nc.tensor.matmul(
                            stB[:, s2 * 512:(s2 + 1) * 512],
                            lhsT=kT_sb[roB:roB + 64, pt, jt * 128:(jt + 1) * 128],
                            rhs=qT_sb[roB:roB + 64, pt, ich * 512:(ich + 1) * 512],
                            start=True, stop=True)
                    ptA = ptp.tile([128, 1024], BF16, name="ptA", tag="pt")
                    ptB = ptp.tile([128, 1024], BF16, name="ptB", tag="pt")
                    nc.scalar.activation(ptA[:, :], stA[:, :],
                                         mybir.ActivationFunctionType.Exp)
                    nc.scalar.activation(ptB[:, :], stB[:, :],
                                         mybir.ActivationFunctionType.Exp)
                    sl = slice(ih * 1024, (ih + 1) * 1024)
                    if jt == 0:
                        nc.vector.tensor_copy(zpA[:, sl], ptA[:, :])
                        nc.vector.tensor_copy(zpB[:, sl], ptB[:, :])
                    else:
                        nc.vector.tensor_add(zpA[:, sl], zpA[:, sl], ptA[:, :])
                        nc.vector.tensor_add(zpB[:, sl], zpB[:, sl], ptB[:, :])
                    vA = v_sb[:, jt, hA * 64:hA * 64 + 64]
                    vB = v_sb[:, jt, hB * 64:hB * 64 + 64]
                    for s2 in range(2):
                        csl = slice(s2 * 512, (s2 + 1) * 512)
                        nc.tensor.matmul(
                            at[roA:roA + 64, csl], lhsT=vA, rhs=ptA[:, csl],
                            start=(jt == 0), stop=(jt == n_jt - 1),
                            skip_group_check=True)
                        nc.tensor.matmul(
                            at[roB:roB + 64, csl], lhsT=vB, rhs=ptB[:, csl],
                            start=(jt == 0), stop=(jt == n_jt - 1),
                            skip_group_check=True)
                    if wi < len(work) and work[wi][0] == "v":
                        emit_work(1)
                    elif jt % 4 == 3:
                        emit_work(1)
                # Z for this i-half: column sums + wide reciprocal + broadcast
                for hh, zp in ((0, zpA), (1, zpB)):
                    cs = stp.tile([1, 1024], FP32, name="cs", tag="st")
                    for s2 in range(2):
                        ich = ih * 2 + s2
                        nc.tensor.matmul(
                            cs[:, s2 * 512:(s2 + 1) * 512],
                            lhsT=ones_sb[:, :],
                            rhs=zp[:, ich * 512:(ich + 1) * 512],
                            start=True, stop=True)
                    nc.vector.tensor_copy(zdance[32 * hh:32 * hh + 1, :],
                                          cs[:, :])
                nc.vector.transpose(zdance_r[:, :], zdance[:, :])
                rview = zdance_r.rearrange("p (b r) -> p b r", r=32)
                nc.vector.reciprocal(rview[:, :, 0:1], rview[:, :, 0:1])
                nc.vector.transpose(zback[:, :], zdance_r[:, :])
                zb = zbp.tile([128, 1024], FP32, name="zb", tag="zb")
                zdA = zdramp.tile([1, 1024], FP32, name="zdA", tag="zdA")
                zdB = zdramp.tile([1, 1024], FP32, name="zdB", tag="zdB")
                nc.sync.dma_start(out=zdA[:, :], in_=zback[0:1, :])
                nc.sync.dma_start(out=zdB[:, :], in_=zback[32:33, :])
                nc.sync.dma_start(out=zb[0:64, :],
                                  in_=zdA[:, :].to_broadcast((64, 1024)))
                nc.sync.dma_start(out=zb[64:128, :],
                                  in_=zdB[:, :].to_broadcast((64, 1024)))
                nc.vector.tensor_mul(atsc[pt][:, ih * 1024:(ih + 1) * 1024],
                                     at[:, :], zb[:, :])
        emit_work(len(work))
        zdramp.release()
        zbp.release()
        zdp.release()
        zpp.release()
        ptp.release()
        qkvp.release()
        atp.release()
        stp.release()
        ph1.release()

        # ---- phase 3: output projection ----
        with tc.tile_pool(name="opp", bufs=2, space="PSUM") as opp, \
             tc.tile_pool(name="osb", bufs=3) as osb:
            for it in range(n // 128):
                o_ps = opp.tile([128, c], FP32, name="o_ps", tag="o")
                for kc in range(kc_w):
                    for cch in range(n_cch):
                        nc.tensor.matmul(
                            o_ps[:, cch * 512:(cch + 1) * 512],
                            lhsT=atsc[kc][:, it * 128:(it + 1) * 128],
                            rhs=woutT_sb[:, kc, cch * 512:(cch + 1) * 512],
                            start=(kc == 0), stop=(kc == kc_w - 1))
                o_sb = osb.tile([128, c], FP32, name="o_sb", tag="osb")
                nc.vector.tensor_copy(o_sb[:, :], o_ps[:, :])
                nc.sync.dma_start(out=out[it * 128:(it + 1) * 128, :],
                                  in_=o_sb[:, :])
        persist.release()

    nc.compile()
    return nc


def make_in_maps(x, Wq, Wk, Wv, Wout, n=2048, c=1024, heads=16, d=64):
    """Shard + pre-transpose + cast inputs for the 8 cores."""
    s = float(d) ** -0.25
    hd_l = (heads // 2) * d
    wT = {}
    for g in range(2):
        sl = slice(g * hd_l, (g + 1) * hd_l)
        wT[g] = (
            np.ascontiguousarray((Wq[sl] * s).T).astype(NP_BF16),
            np.ascontiguousarray((Wk[sl] * s).T).astype(NP_BF16),
            np.ascontiguousarray(Wv[sl].T).astype(NP_BF16),
            np.ascontiguousarray(Wout.T[sl]).astype(NP_BF16),
        )
    in_maps = []
    for core in range(N_CORES):
        b = core // 2
        g = core % 2
        wq, wk, wv, wo = wT[g]
        in_maps.append({
            "xT": np.ascontiguousarray(x[b].T).astype(NP_BF16),
            "wqT": wq, "wkT": wk, "wvT": wv, "woutT": wo,
        })
    return in_maps


_NC_CACHE = {}


def kernel(x, Wq, Wk, Wv, Wout, bout):
    global LAST_EXEC_TIME_NS
    b, n, c = x.shape
    heads = 16
    d = 64
    hd_l = (heads // 2) * d

    if "nc" not in _NC_CACHE:
        _NC_CACHE["nc"] = build_nc(n=n, c=c, hd_l=hd_l)
    nc = _NC_CACHE["nc"]

    in_maps = make_in_maps(np.asarray(x, np.float32), np.asarray(Wq, np.float32),
                           np.asarray(Wk, np.float32), np.asarray(Wv, np.float32),
                           np.asarray(Wout, np.float32), n=n, c=c, heads=heads, d=d)

    profile = os.environ.get("BASS_KERNEL_PROFILE", "0") == "1"
    if profile:
        _install_ntff_shim()
    res = bass_utils.run_bass_kernel_spmd(
        nc, in_maps, core_ids=list(range(N_CORES)), trace=profile)
    LAST_EXEC_TIME_NS = res.exec_time_ns

    bout = np.asarray(bout, np.float32)
    out = np.empty((b, n, c), np.float32)
    for bb in range(b):
        out[bb] = res.results[2 * bb]["out"] + res.results[2 * bb + 1]["out"] + bout
    return out


# revision 14
# speedup vs baseline: 2.0722x; 1.0249x over previous
"""Self-attention (CrossAttention with context=x) Bass kernel for Trainium2, 8 NeuronCores.

Problem: x:(4,2048,1024) fp32, 16 heads x 64 dim, Wq/Wk/Wv:(1024,1024), Wout:(1024,1024), bout:(1024,)
  q = x@WqT, k = x@WkT, v = x@WvT  (per head d=64, scale d**-0.25 on q and k)
  weight = softmax(q k^T), a = weight @ v, out = a@WoutT + bout

Sharding: core j handles batch j//2, head-group j%2 (8 of 16 heads).
Each core computes a partial output (its heads' contribution to out[batch]);
the host sums core pairs and adds bout.

V3 design (per core, bf16 compute, fp32 accumulate):
- Head-PAIR processing: scores row-tiled on the PE array (head A rows 0-63,
  head B rows 64-127, concurrent K=64 matmuls); values col-tiled (A cols 0-63,
  B cols 64-127, concurrent M=64 matmuls).
- exp on ScalarE from PSUM in [128,1024] tiles, double-buffered per head so
  ScalarE stays saturated. No max-subtraction (logits are small by construction).
- Softmax denominators: Zpart[p,i] += PT chunks on VectorE (bf16), column-summed
  by a ones-vector matmul; reciprocal computed in a 32x32-transposed layout so it
  runs wide across partitions; broadcast to rows via GpSimd partition_broadcast;
  normalization folded into a single PSUM->SBUF multiply per pair.
- Output projection from normalized a^T tiles; host adds pair partials + bout.
"""
import os
import sys
import types
import numpy as np

import concourse.bass as bass
import concourse.bacc as bacc
import concourse.tile as tile
from concourse import mybir
from concourse import bass_utils

BF16 = mybir.dt.bfloat16
FP32 = mybir.dt.float32
NP_BF16 = mybir.dt.np(BF16)

N_CORES = 8
LAST_EXEC_TIME_NS = None


def _install_ntff_shim():
    """Shim for missing antenv.axon_hooks so trace=True can capture NTFF profiles."""
    if "antenv.axon_hooks" in sys.modules:
        return
    try:
        import antenv  # noqa: F401
        from trn_agent_boot.trn_boot import _ntff_profile_via_ctypes
    except Exception:
        return
    hook = _ntff_profile_via_ctypes("/opt/axon/libaxon_pjrt.so")
    mod = types.ModuleType("antenv.axon_hooks")
    mod._hook = hook
    mod.set_axon_ntff_profile_hook = lambda h: setattr(mod, "_hook", h)
    mod.get_axon_ntff_profile_hook = lambda: mod._hook
    sys.modules["antenv.axon_hooks"] = mod
    sys.modules["antenv"].axon_hooks = mod


def build_nc(n=2048, c=1024, hd_l=512, num_devices=N_CORES, debug=False):
    """Build the per-core SPMD Bass graph.

    n: sequence length, c: model dim, hd_l: local head-dims (heads_l*64).
    """
    d = 64
    heads_l = hd_l // d
    pairs = heads_l // 2
    kc_x = c // 128      # contraction chunks over c
    mt_q = hd_l // 128   # qT/kT partition tiles (one per head pair)
    n_ich = n // 512     # i chunks of 512
    n_jt = n // 128      # j tiles of 128
    kc_w = hd_l // 128   # out-proj contraction chunks
    n_cch = c // 512     # out-proj N chunks
    n_ih = n // 1024     # i halves

    nc = bacc.Bacc("TRN2", target_bir_lowering=False, debug=False,
                   num_devices=num_devices)
    xT = nc.dram_tensor("xT", [c, n], BF16, kind="ExternalInput").ap()
    wqT = nc.dram_tensor("wqT", [c, hd_l], BF16, kind="ExternalInput").ap()
    wkT = nc.dram_tensor("wkT", [c, hd_l], BF16, kind="ExternalInput").ap()
    wvT = nc.dram_tensor("wvT", [c, hd_l], BF16, kind="ExternalInput").ap()
    woutT = nc.dram_tensor("woutT", [hd_l, c], BF16, kind="ExternalInput").ap()
    out = nc.dram_tensor("out", [n, c], FP32, kind="ExternalOutput").ap()
    if debug:
        dbg_at = nc.dram_tensor("dbg_at", [128, n], FP32, kind="ExternalOutput").ap()
        dbg_zp = nc.dram_tensor("dbg_zp", [128, n], FP32, kind="ExternalOutput").ap()
        dbg_zd = nc.dram_tensor("dbg_zd", [64, n], FP32, kind="ExternalOutput").ap()
        dbg_zback = nc.dram_tensor("dbg_zback", [64, n], FP32, kind="ExternalOutput").ap()
        dbg_zb = nc.dram_tensor("dbg_zb", [128, n], FP32, kind="ExternalOutput").ap()

    xT_r = xT.rearrange("(kc p) n -> kc p n", p=128)
    wqT_r = wqT.rearrange("(kc p) h -> kc p h", p=128)
    wkT_r = wkT.rearrange("(kc p) h -> kc p h", p=128)
    wvT_r = wvT.rearrange("(kc p) h -> kc p h", p=128)
    woutT_r = woutT.rearrange("(kc p) c2 -> kc p c2", p=128)

    with tile.TileContext(nc) as tc:
        persist = tc.alloc_tile_pool(name="persist", bufs=1)
        qT_sb = persist.tile([128, mt_q, n], BF16, name="qT_sb")
        kT_sb = persist.tile([128, mt_q, n], BF16, name="kT_sb")
        v_sb = persist.tile([128, n_jt, hd_l], BF16, name="v_sb")
        woutT_sb = persist.tile([128, kc_w, c], BF16, name="woutT_sb")
        atsc = [persist.tile([128, n], BF16, name=f"atsc{i}", tag=f"atsc{i}")
                for i in range(mt_q)]
        ones_sb = persist.tile([128, 1], BF16, name="ones_sb")
        nc.vector.memset(ones_sb[:, :], 1.0)

        ph1 = tc.alloc_tile_pool(name="ph1", bufs=1)
        xT_sb = ph1.tile([128, kc_x, n], BF16, name="xT_sb")
        wqT_sb = ph1.tile([128, kc_x, hd_l], BF16, name="wqT_sb")
        wkT_sb = ph1.tile([128, kc_x, hd_l], BF16, name="wkT_sb")
        wvT_sb = ph1.tile([128, kc_x, hd_l], BF16, name="wvT_sb")

        for kc in range(kc_x):
            nc.sync.dma_start(out=xT_sb[:, kc, :], in_=xT_r[kc])
            nc.sync.dma_start(out=wqT_sb[:, kc, :], in_=wqT_r[kc])
            nc.sync.dma_start(out=wkT_sb[:, kc, :], in_=wkT_r[kc])
            nc.sync.dma_start(out=wvT_sb[:, kc, :], in_=wvT_r[kc])
        for kc in range(kc_w):
            nc.sync.dma_start(out=woutT_sb[:, kc, :], in_=woutT_r[kc])

        # ---- phases 1+2 interleaved: qkv chains feed the attention pipeline ----
        # Attention is ScalarE-bound (exp). qkv matmul chains are emitted inside
        # the attention loop so the PE computes projections while ScalarE exps.
        stp = tc.alloc_tile_pool(name="stp", bufs=2, space="PSUM")
        atp = tc.alloc_tile_pool(name="atp", bufs=1, space="PSUM")
        qkvp = tc.alloc_tile_pool(name="qkvp", bufs=2, space="PSUM")
        ptp = tc.alloc_tile_pool(name="ptp", bufs=14)
        zpp = tc.alloc_tile_pool(name="zpp", bufs=2)
        zdp = tc.alloc_tile_pool(name="zdp", bufs=2)
        zbp = tc.alloc_tile_pool(name="zbp", bufs=2)
        zdramp = tc.alloc_tile_pool(name="zdramp", bufs=2, space="DRAM")

        def qk_chain(wsb, dst, mt, ip):
            """One q/k projection chain: 2 psum tiles, LDW amortized x2."""
            ps = [qkvp.tile([128, 512], FP32, name="ps", tag="qkv")
                  for _ in range(2)]
            for kc in range(kc_x):
                for i2 in range(2):
                    ich = ip * 2 + i2
                    nc.tensor.matmul(
                        ps[i2][:, :],
                        lhsT=wsb[:, kc, mt * 128:(mt + 1) * 128],
                        rhs=xT_sb[:, kc, ich * 512:(ich + 1) * 512],
                        start=(kc == 0), stop=(kc == kc_x - 1))
            for i2 in range(2):
                ich = ip * 2 + i2
                nc.vector.tensor_copy(
                    dst[:, mt, ich * 512:(ich + 1) * 512], ps[i2][:, :])

        def v_chain(nt):
            v_ps = qkvp.tile([128, hd_l], FP32, name="v_ps", tag="qkv")
            for kc in range(kc_x):
                nc.tensor.matmul(
                    v_ps[:, :],
                    lhsT=xT_sb[:, kc, nt * 128:(nt + 1) * 128],
                    rhs=wvT_sb[:, kc, :],
                    start=(kc == 0), stop=(kc == kc_x - 1))
            nc.vector.tensor_copy(v_sb[:, nt, :], v_ps[:, :])

        # work list of deferred qkv chains, emitted inside the attention loop
        work = []
        for ip in range(1, n_ich // 2):
            work.append(("k", 0, ip))
        for nt in range(2, n_jt):
            work.append(("v", nt))
        for ip in range(1, n_ich // 2):
            work.append(("q", 0, ip))
        for mt in range(1, mt_q):
            for ip in range(n_ich // 2):
                work.append(("q", mt, ip))
                work.append(("k", mt, ip))
        wi = 0

        def emit_work(k_items):
            nonlocal wi
            for _ in range(k_items):
                if wi >= len(work):
                    return
                item = work[wi]
                wi += 1
                if item[0] == "v":
                    v_chain(item[1])
                else:
                    wsb, dst = ((wqT_sb, qT_sb) if item[0] == "q"
                                else (wkT_sb, kT_sb))
                    qk_chain(wsb, dst, item[1], item[2])

        # prime: only what the first scores need (q/k i-cols 0:1024, v tiles 0-1)
        qk_chain(wqT_sb, qT_sb, 0, 0)
        qk_chain(wkT_sb, kT_sb, 0, 0)
        v_chain(0)
        v_chain(1)

        zdance = zdp.tile([64, 1024], FP32, name="zdance", tag="zd")
        zdance_r = zdp.tile([64, 1024], FP32, name="zdance_r", tag="zdr")
        zback = zdp.tile([64, 1024], FP32, name="zback", tag="zbk")
        nc.vector.memset(zdance[:, :], 1.0)
        nc.vector.memset(zdance_r[:, :], 1.0)

        for pt in range(pairs):
            roA, roB = 0, 64
            hA, hB = 2 * pt, 2 * pt + 1
            zpA = zpp.tile([128, n], BF16, name="zpA", tag="zpA")
            zpB = zpp.tile([128, n], BF16, name="zpB", tag="zpB")
            for ih in range(n_ih):
                at = atp.tile([128, 1024], FP32, name="at", tag="at")
                for jt in range(n_jt):
                    stA = stp.tile([128, 1024], FP32, name="stA", tag="st")
                    stB = stp.tile([128, 1024], FP32, name="stB", tag="st")
                    for s2 in range(2):
                        ich = ih * 2 + s2
                        nc.tensor.matmul(
                            stA[:, s2 * 512:(s2 + 1) * 512],
                            lhsT=kT_sb[roA:roA + 64, pt, jt * 128:(jt + 1) * 128],
                            rhs=qT_sb[roA:roA + 64, pt, ich * 512:(ich + 1) * 512],
                            start=True, stop=True)
                        nc.tensor.matmul(
                            stB[:, s2 * 512:(s2 + 1) * 512],
                            lhsT=kT_sb[roB:roB + 64, pt, jt * 128:(jt + 1) * 128],
                            rhs=qT_sb[roB:roB + 64, pt, ich * 512:(ich + 1) * 512],
                            start=True, stop=True)
                    ptA = ptp.tile([128, 1024], BF16, name="ptA", tag="pt")
                    ptB = ptp.tile([128, 1024], BF16, name="ptB", tag="pt")
                    nc.scalar.activation(ptA[:, :], stA[:, :],
                                         mybir.ActivationFunctionType.Exp)
                    nc.scalar.activation(ptB[:, :], stB[:, :],
                                         mybir.ActivationFunctionType.Exp)
                    sl = slice(ih * 1024, (ih + 1) * 1024)
                    if jt == 0:
                        nc.vector.tensor_copy(zpA[:, sl], ptA[:, :])
                        nc.vector.tensor_copy(zpB[:, sl], ptB[:, :])
                    else:
                        nc.vector.tensor_add(zpA[:, sl], zpA[:, sl], ptA[:, :])
                        nc.vector.tensor_add(zpB[:, sl], zpB[:, sl], ptB[:, :])
                    vA = v_sb[:, jt, hA * 64:hA * 64 + 64]
                    vB = v_sb[:, jt, hB * 64:hB * 64 + 64]
                    for s2 in range(2):
                        csl = slice(s2 * 512, (s2 + 1) * 512)
                        nc.tensor.matmul(
                            at[roA:roA + 64, csl], lhsT=vA, rhs=ptA[:, csl],
                            start=(jt == 0), stop=(jt == n_jt - 1),
                            skip_group_check=True)
                        nc.tensor.matmul(
                            at[roB:roB + 64, csl], lhsT=vB, rhs=ptB[:, csl],
                            start=(jt == 0), stop=(jt == n_jt - 1),
                            skip_group_check=True)
                    if wi < len(work) and (pt, ih) == (0, 0) \
                            and (work[wi][0] == "v" or work[wi][1] == 0):
                        emit_work(1)
                    elif jt % 4 == 3:
                        emit_work(1)
                # Z for this i-half: column sums + wide reciprocal + broadcast
                for hh, zp in ((0, zpA), (1, zpB)):
                    for s2 in range(2):
                        ich = ih * 2 + s2
                        cs = qkvp.tile([1, 512], FP32, name="cs", tag="qkv")
                        nc.tensor.matmul(
                            cs[:, :],
                            lhsT=ones_sb[:, :],
                            rhs=zp[:, ich * 512:(ich + 1) * 512],
                            start=True, stop=True)
                        nc.vector.tensor_copy(
                            zdance[32 * hh:32 * hh + 1,
                                   s2 * 512:(s2 + 1) * 512],
                            cs[:, :])
                nc.vector.transpose(zdance_r[:, :], zdance[:, :])
                rview = zdance_r.rearrange("p (b r) -> p b r", r=32)
                nc.vector.reciprocal(rview[:, :, 0:1], rview[:, :, 0:1])
                nc.vector.transpose(zback[:, :], zdance_r[:, :])
                zb = zbp.tile([128, 1024], FP32, name="zb", tag="zb")
                zdA = zdramp.tile([1, 1024], FP32, name="zdA", tag="zdA")
                zdB = zdramp.tile([1, 1024], FP32, name="zdB", tag="zdB")
                nc.sync.dma_start(out=zdA[:, :], in_=zback[0:1, :])
                nc.sync.dma_start(out=zdB[:, :], in_=zback[32:33, :])
                nc.sync.dma_start(out=zb[0:64, :],
                                  in_=zdA[:, :].to_broadcast((64, 1024)))
                nc.sync.dma_start(out=zb[64:128, :],
                                  in_=zdB[:, :].to_broadcast((64, 1024)))
                nc.vector.tensor_mul(atsc[pt][:, ih * 1024:(ih + 1) * 1024],
                                     at[:, :], zb[:, :])
        emit_work(len(work))
        zdramp.release()
        zbp.release()
        zdp.release()
        zpp.release()
        ptp.release()
        qkvp.release()
        atp.release()
        stp.release()
        ph1.release()

        # ---- phase 3: output projection ----
        with tc.tile_pool(name="opp", bufs=2, space="PSUM") as opp, \
             tc.tile_pool(name="osb", bufs=3) as osb:
            for it in range(n // 128):
                o_ps = opp.tile([128, c], FP32, name="o_ps", tag="o")
                for kc in range(kc_w):
                    for cch in range(n_cch):
                        nc.tensor.matmul(
                            o_ps[:, cch * 512:(cch + 1) * 512],
                            lhsT=atsc[kc][:, it * 128:(it + 1) * 128],
                            rhs=woutT_sb[:, kc, cch * 512:(cch + 1) * 512],
                            start=(kc == 0), stop=(kc == kc_w - 1))
                o_sb = osb.tile([128, c], FP32, name="o_sb", tag="osb")
                nc.vector.tensor_copy(o_sb[:, :], o_ps[:, :])
                nc.sync.dma_start(out=out[it * 128:(it + 1) * 128, :],
                                  in_=o_sb[:, :])
        persist.release()

    nc.compile()
    return nc


def make_in_maps(x, Wq, Wk, Wv, Wout, n=2048, c=1024, heads=16, d=64):
    """Shard + pre-transpose + cast inputs for the 8 cores."""
    s = float(d) ** -0.25
    hd_l = (heads // 2) * d
    wT = {}
    for g in range(2):
        sl = slice(g * hd_l, (g + 1) * hd_l)
        wT[g] = (
            np.ascontiguousarray((Wq[sl] * s).T).astype(NP_BF16),
            np.ascontiguousarray((Wk[sl] * s).T).astype(NP_BF16),
            np.ascontiguousarray(Wv[sl].T).astype(NP_BF16),
            np.ascontiguousarray(Wout.T[sl]).astype(NP_BF16),
        )
    in_maps = []
    for core in range(N_CORES):
        b = core // 2
        g = core % 2
        wq, wk, wv, wo = wT[g]
        in_maps.append({
            "xT": np.ascontiguousarray(x[b].T).astype(NP_BF16),
            "wqT": wq, "wkT": wk, "wvT": wv, "woutT": wo,
        })
    return in_maps


_NC_CACHE = {}


def kernel(x, Wq, Wk, Wv, Wout, bout):
    global LAST_EXEC_TIME_NS
    b, n, c = x.shape
    heads = 16
    d = 64
    hd_l = (heads // 2) * d

    if "nc" not in _NC_CACHE:
        _NC_CACHE["nc"] = build_nc(n=n, c=c, hd_l=hd_l)
    nc = _NC_CACHE["nc"]

    in_maps = make_in_maps(np.asarray(x, np.float32), np.asarray(Wq, np.float32),
                           np.asarray(Wk, np.float32), np.asarray(Wv, np.float32),
                           np.asarray(Wout, np.float32), n=n, c=c, heads=heads, d=d)

    profile = os.environ.get("BASS_KERNEL_PROFILE", "0") == "1"
    if profile:
        _install_ntff_shim()
    res = bass_utils.run_bass_kernel_spmd(
        nc, in_maps, core_ids=list(range(N_CORES)), trace=profile)
    LAST_EXEC_TIME_NS = res.exec_time_ns

    bout = np.asarray(bout, np.float32)
    out = np.empty((b, n, c), np.float32)
    for bb in range(b):
        out[bb] = res.results[2 * bb]["out"] + res.results[2 * bb + 1]["out"] + bout
    return out
